# revision 1
# baseline (speedup 1.0000x reference)
"""Causal self-attention Trainium2 Bass kernel.

Reference (hardcoded):
    N_EMBD=1024, N_HEAD=16, B=4, T=2048, hd=64
    qkv = x @ W_attn.T ; q,k,v split
    att = softmax(mask(q k^T * 8))          # note: reference MULTIPLIES by sqrt(hd)
    y   = (att @ v) reassembled ; out = y @ W_proj.T + b_proj

Sharding over 8 cores: core = (b, hg) with b = core//2 in 0..3 (batch),
hg = core%2 (head-group of 8 heads). Each core computes the partial c_proj
output for its 8 heads of its batch; host adds the two per-batch partials
and the bias.

Per-core kernel layout strategy (all matmul operands viewed as float32r):
  xT   [1024, 2048]   x[b]^T                  (c on partitions)
  qT/kT pair tiles [128, 2048] x4: rows 0-63 head 2a, 64-127 head 2a+1 ([d,t])
  v    natural [t, d] in 16 tiles [128, 8*65] with a ones column per head
  pass1: S[tq,tk] blocks via 2-head row-packed K=64 matmuls -> row maxes
         (causal mask applied by accumulating ident.T @ maskU = -1e30 pattern)
  pass2: S'[tk,tq] blocks via K=65 matmuls (kaug row64=ones, qaug row64=-max)
         -> exp on ScalarE -> p^T in SBUF
  PV:    psum_y[65, 512] += v_aug^T.T @ p^T  (row 64 = softmax sums, free)
  norm:  reciprocal of sums, partition-broadcast, multiply
  proj:  outT[o,t] = wpT.T @ yT  accumulated over 4 c-tiles
"""

import math
from contextlib import ExitStack

import numpy as np

import concourse.bass as bass
import concourse.bacc as bacc
import concourse.mybir as mybir
import concourse.tile as tile

F32 = mybir.dt.float32
F32R = mybir.dt.float32r
AF = mybir.ActivationFunctionType
AX = mybir.AxisListType
ALU = mybir.AluOpType

NEG_BIG = -30000.0


def _R(ap):
    return ap.bitcast(F32R)


def build_nc(T=2048, CK=1024, NH=8):
    """Emit the per-core program. T: seq len, CK: embed dim (contraction),
    NH: heads on this core (head dim fixed 64)."""
    HD = 64
    NP = NH // 2          # head pairs
    CL = NH * HD          # core-local channels (512)
    NT = T // 128         # tq/tk tiles
    NS = T // 512         # 512-wide slices
    NC = CK // 128        # contraction tiles
    THT = min(T, 1024)    # t-chunk for phase P x residency
    NHALF = T // THT

    nc = bacc.Bacc(None, target_bir_lowering=False)

    xT = nc.declare_dram_parameter("xT", [CK, T], F32, isOutput=False)
    wqT = nc.declare_dram_parameter("wqT", [CK, CL], F32, isOutput=False)
    wkT = nc.declare_dram_parameter("wkT", [CK, CL], F32, isOutput=False)
    wvT = nc.declare_dram_parameter("wvT", [CK, CL], F32, isOutput=False)
    wpT = nc.declare_dram_parameter("wpT", [CL, CK], F32, isOutput=False)
    maskU = nc.declare_dram_parameter("maskU", [128, 896], F32, isOutput=False)
    maskL = nc.declare_dram_parameter("maskL", [128, 896], F32, isOutput=False)
    ident = nc.declare_dram_parameter("ident", [128, 128], F32, isOutput=False)
    ones_row = nc.declare_dram_parameter("ones_row", [1, T], F32, isOutput=False)
    ones_va = nc.declare_dram_parameter("ones_va", [128, NH * 65], F32, isOutput=False)
    zeros_row = nc.declare_dram_parameter("zeros_row", [1, T], F32, isOutput=False)
    outT = nc.declare_dram_parameter("outT", [CK, T], F32, isOutput=True)

    with tile.TileContext(nc) as tc, ExitStack() as ctx:
        singles = ctx.enter_context(tc.tile_pool(name="singles", bufs=1))

        ident_sb = singles.tile([128, 128], F32, tag="ident")
        nc.sync.dma_start(out=ident_sb, in_=ident[:, :])
        identR_sb = singles.tile([128, 128], F32, tag="identR")
        nc.sync.dma_start(out=_R(identR_sb), in_=_R(ident[:, :]))
        maskU_sb = singles.tile([128, 896], F32, tag="maskU")
        nc.sync.dma_start(out=_R(maskU_sb), in_=_R(maskU[:, :]))
        maskL_sb = singles.tile([128, 896], F32, tag="maskL")
        nc.sync.dma_start(out=_R(maskL_sb), in_=_R(maskL[:, :]))

        # resident activations
        qt = [singles.tile([128, T], F32, tag=f"qt{a}", name=f"qt{a}") for a in range(NP)]
        kt = [singles.tile([128, T], F32, tag=f"kt{a}", name=f"kt{a}") for a in range(NP)]
        va = [singles.tile([128, NH * 65], F32, tag=f"va{i}", name=f"va{i}") for i in range(NT)]
        maxtile = singles.tile([NH, T], F32, tag="maxtile")
        ones4 = singles.tile([1, 64], F32, tag="ones4")
        nc.sync.dma_start(out=_R(ones4), in_=_R(ones_row[:, 0:64]))

        for i in range(NT):
            nc.sync.dma_start(out=_R(va[i][:, :]), in_=_R(ones_va[:, :]))

        # ---------------- phase P: projections ----------------
        # P1: q/k (wq+wk resident), P2: v (wv resident); x streamed per t-chunk
        with tc.tile_pool(name="ppj", bufs=2, space="PSUM") as ppj:
            with tc.tile_pool(name="wpool1", bufs=1) as wpool, \
                 tc.tile_pool(name="xpool1", bufs=NC) as xpool:
                wq_sb = [wpool.tile([128, CL], F32, tag=f"wq{c}", name=f"wq{c}")
                         for c in range(NC)]
                wk_sb = [wpool.tile([128, CL], F32, tag=f"wk{c}", name=f"wk{c}")
                         for c in range(NC)]
                for c in range(NC):
                    nc.sync.dma_start(out=_R(wq_sb[c]), in_=_R(wqT[128 * c:128 * (c + 1), :]))
                    nc.sync.dma_start(out=_R(wk_sb[c]), in_=_R(wkT[128 * c:128 * (c + 1), :]))
                for half in range(NHALF):
                    t0 = half * THT
                    xh = [xpool.tile([128, THT], F32, tag="xh", name=f"xh{half}_{c}")
                          for c in range(NC)]
                    for c in range(NC):
                        nc.sync.dma_start(
                            out=_R(xh[c]), in_=_R(xT[128 * c:128 * (c + 1), t0:t0 + THT]))
                    for sl in range(THT // 512):
                        s = (t0 // 512) + sl
                        for a in range(NP):
                            for w_sb, dest in ((wq_sb, qt), (wk_sb, kt)):
                                ps = ppj.tile([128, 512], F32, tag="pj", name="pj_qk")
                                for c in range(NC):
                                    nc.tensor.matmul(
                                        ps, _R(w_sb[c][:, 128 * a:128 * (a + 1)]),
                                        _R(xh[c][:, 512 * sl:512 * (sl + 1)]),
                                        start=(c == 0), stop=(c == NC - 1))
                                nc.scalar.copy(_R(dest[a][:, 512 * s:512 * (s + 1)]), ps)

            with tc.tile_pool(name="wpool2", bufs=1) as wpool, \
                 tc.tile_pool(name="xpool2", bufs=NC) as xpool:
                wv_sb = [wpool.tile([128, CL], F32, tag=f"wv{c}", name=f"wv{c}")
                         for c in range(NC)]
                for c in range(NC):
                    nc.sync.dma_start(out=_R(wv_sb[c]), in_=_R(wvT[128 * c:128 * (c + 1), :]))
                for half in range(NHALF):
                    t0 = half * THT
                    xh = [xpool.tile([128, THT], F32, tag="xh", name=f"xv{half}_{c}")
                          for c in range(NC)]
                    for c in range(NC):
                        nc.sync.dma_start(
                            out=_R(xh[c]), in_=_R(xT[128 * c:128 * (c + 1), t0:t0 + THT]))
                    for il in range(THT // 128):
                        i = t0 // 128 + il
                        ps = ppj.tile([128, CL], F32, tag="pj", name="pj_v")
                        for c in range(NC):
                            nc.tensor.matmul(
                                ps, _R(xh[c][:, 128 * il:128 * (il + 1)]), _R(wv_sb[c]),
                                start=(c == 0), stop=(c == NC - 1))
                        va_view = va[i].rearrange("p (h e) -> p h e", e=65)
                        ps_view = ps.rearrange("p (h e) -> p h e", e=HD)
                        nc.vector.tensor_copy(_R(va_view[:, :, 0:HD]), ps_view)

        # ---------------- phase A: attention ----------------
        ytpool = ctx.enter_context(tc.tile_pool(name="ytpool", bufs=1))
        yt = [ytpool.tile([128, T], F32, tag=f"yt{a}", name=f"yt{a}") for a in range(NP)]
        with tc.tile_pool(name="p1ps", bufs=3, space="PSUM") as p1ps, \
             tc.tile_pool(name="p2ps", bufs=2, space="PSUM") as p2ps, \
             tc.tile_pool(name="yvps", bufs=2, space="PSUM") as yvps, \
             tc.tile_pool(name="augs", bufs=2) as augs, \
             tc.tile_pool(name="ptp", bufs=3) as ptp, \
             tc.tile_pool(name="small", bufs=2) as small:

            def head_setup(h):
                """Build per-head K=65 augmented tiles. kaug row64 = ones;
                qaug row64 = 0 for pass-1, overwritten with -max for pass-2."""
                a, hip = h // 2, h % 2
                kaug = augs.tile([65, T], F32, tag="kaug", name=f"kaug{h}")
                nc.sync.dma_start(out=_R(kaug[0:64, :]),
                                  in_=_R(kt[a][64 * hip:64 * hip + 64, :]))
                nc.sync.dma_start(out=_R(kaug[64:65, :]), in_=_R(ones_row[:, :]))
                qaug = augs.tile([65, T], F32, tag="qaug", name=f"qaug{h}")
                nc.sync.dma_start(out=_R(qaug[0:64, :]),
                                  in_=_R(qt[a][64 * hip:64 * hip + 64, :]))
                nc.sync.dma_start(out=_R(qaug[64:65, :]), in_=_R(zeros_row[:, :]))
                sumt = small.tile([NS, 512], F32, tag="sumt", bufs=1, name="sumt")
                rinv = small.tile([NS, 512], F32, tag="rinv", bufs=1, name="rinv")
                ytmp = [small.tile([65, 512], F32, tag=f"ytmp{s}",
                                   name=f"ytmp{h}_{s}", bufs=1) for s in range(NS)]
                return dict(h=h, kaug=kaug, qaug=qaug, sumt=sumt, rinv=rinv,
                            ytmp=ytmp)

            def p1_row(st, i):
                """Pass-1 row i for head h: causal-trimmed S blocks via K=65
                matmuls in [tq, tk] layout; -max of each row into maxtile."""
                h, kaug, qaug = st["h"], st["kaug"], st["qaug"]
                jd, m = i // 4, i % 4
                W = 128 * (m + 1)            # diag block valid width
                mA = small.tile([128, 4], F32, tag="mA", name="mA")
                for j in range(jd + 1):
                    diag = (j == jd)
                    w = W if diag else 512
                    psA = p1ps.tile([128, 512], F32, tag="blk", name="psA")
                    nc.tensor.matmul(
                        psA[:, 0:w], _R(qaug[:, 128 * i:128 * (i + 1)]),
                        _R(kaug[:, 512 * j:512 * j + w]),
                        start=True, stop=not diag)
                    if diag:
                        # boundary 128-chunk gets the pure triangular mask
                        msl = _R(maskU_sb[:, 384:512])
                        nc.tensor.matmul(psA[:, W - 128:W], _R(identR_sb), msl,
                                         start=False, stop=True)
                    nc.vector.reduce_max(mA[:, j:j + 1], psA[:, 0:w], axis=AX.X)
                negmax = small.tile([128, 1], F32, tag="negmax", name="negmax")
                nc.vector.tensor_reduce(
                    negmax, mA[:, 0:jd + 1], axis=AX.X, op=ALU.max, negate=True)
                tp = p1ps.tile([1, 128], F32, tag="tp", bufs=1, name="tp")
                nc.tensor.transpose(tp, negmax, ident_sb)
                tps = small.tile([1, 128], F32, tag="tps", name="tps")
                nc.vector.tensor_copy(tps, tp)
                nc.sync.dma_start(
                    out=maxtile[h:h + 1, 128 * i:128 * (i + 1)], in_=tps)

            def p1_done(st):
                # all pass-1 reads of qaug row64 are finished; load -max for pass-2
                h = st["h"]
                nc.sync.dma_start(out=_R(st["qaug"][64:65, :]),
                                  in_=_R(maxtile[h:h + 1, :]))

            def a2_slice(st, s):
                """Pass-2 + PV for (head, tq-slice s), causal-trimmed."""
                h, kaug, qaug = st["h"], st["kaug"], st["qaug"]
                a, hip = h // 2, h % 2
                nts = 4 * s + 4
                yps = yvps.tile([65, 512], F32, tag="y", name="yps")
                for t in range(nts):
                    mp = t - 4 * s
                    c0 = 128 * mp if mp > 0 else 0   # first valid column
                    ps2 = p2ps.tile([128, 512], F32, tag="s2", name="ps2")
                    nc.tensor.matmul(
                        ps2[:, c0:512], _R(kaug[:, 128 * t:128 * (t + 1)]),
                        _R(qaug[:, 512 * s + c0:512 * (s + 1)]),
                        start=True, stop=(mp < 0))
                    if mp >= 0:
                        msl = _R(maskL_sb[:, 384:512])
                        nc.tensor.matmul(ps2[:, c0:c0 + 128], _R(identR_sb), msl,
                                         start=False, stop=True)
                    pt = ptp.tile([128, 512], F32, tag="pt", name="pt")
                    nc.scalar.activation(_R(pt[:, c0:512]), ps2[:, c0:512], AF.Exp)
                    nc.tensor.matmul(
                        yps[:, c0:512], _R(va[t][:, 65 * h:65 * h + 65]),
                        _R(pt[:, c0:512]),
                        start=(t == 0), stop=(t == nts - 1),
                        skip_group_check=True)
                nc.scalar.copy(st["ytmp"][s], yps)
                nc.sync.dma_start(out=st["sumt"][s:s + 1, :],
                                  in_=st["ytmp"][s][64:65, :])

            def a2_finish(st):
                h = st["h"]
                a, hip = h // 2, h % 2
                sumt, rinv, ytmp = st["sumt"], st["rinv"], st["ytmp"]
                nc.vector.reciprocal(rinv, sumt)
                for s in range(NS):
                    rr = small.tile([1, 512], F32, tag="rr", bufs=2, name="rr")
                    nc.sync.dma_start(out=_R(rr), in_=_R(rinv[s:s + 1, :]))
                    rb = yvps.tile([64, 512], F32, tag="y", name="rb")
                    nc.tensor.matmul(
                        rb, _R(ones4[0:1, :]), _R(rr[0:1, :]),
                        start=True, stop=True)
                    if hip == 0:
                        nc.vector.tensor_mul(
                            _R(yt[a][0:64, 512 * s:512 * (s + 1)]), ytmp[s][0:64, :], rb)
                    else:
                        yn = small.tile([64, 512], F32, tag="yn", bufs=1, name="yn")
                        nc.vector.tensor_mul(yn, ytmp[s][0:64, :], rb)
                        nc.sync.dma_start(
                            out=_R(yt[a][64:128, 512 * s:512 * (s + 1)]), in_=_R(yn))

            # software pipeline at head granularity: pass-1 of head h runs
            # (PE-light, DVE-heavy) interleaved with pass-2/PV of head h-1
            # (PE-heavy) so both engines stay fed.
            def a2_sections(st):
                for s in range(NS):
                    yield lambda st=st, s=s: a2_slice(st, s)
                yield lambda st=st: a2_finish(st)

            sts = [None] * NH
            for h in range(NH + 1):
                prev_iter = a2_sections(sts[h - 1]) if h >= 1 else iter(())
                if h < NH:
                    sts[h] = head_setup(h)
                    k = 0
                    for i in range(NT):
                        p1_row(sts[h], i)
                        if i % 3 == 2:
                            nxt = next(prev_iter, None)
                            if nxt is not None:
                                nxt()
                    p1_done(sts[h])
                for nxt in prev_iter:
                    nxt()

        # ---------------- phase PR: output projection ----------------
        with tc.tile_pool(name="wppool", bufs=1) as wppool, \
             tc.tile_pool(name="prps", bufs=2, space="PSUM") as prps, \
             tc.tile_pool(name="stg", bufs=3) as stg:
            wp_sb = [wppool.tile([128, CK], F32, tag=f"wp{t_}", name=f"wp{t_}")
                     for t_ in range(NP)]
            for t_ in range(NP):
                nc.sync.dma_start(out=_R(wp_sb[t_]), in_=_R(wpT[128 * t_:128 * (t_ + 1), :]))
            for o in range(NC):
                for s in range(NS):
                    ps = prps.tile([128, 512], F32, tag="pr")
                    for t_ in range(NP):
                        nc.tensor.matmul(
                            ps, _R(wp_sb[t_][:, 128 * o:128 * (o + 1)]),
                            _R(yt[t_][:, 512 * s:512 * (s + 1)]),
                            start=(t_ == 0), stop=(t_ == NP - 1))
                    st = stg.tile([128, 512], F32, tag="st")
                    nc.scalar.copy(st, ps)
                    nc.sync.dma_start(
                        out=outT[128 * o:128 * (o + 1), 512 * s:512 * (s + 1)], in_=st)

    nc.finalize()
    return nc


def make_masks():
    r = np.arange(128)[:, None]
    c = np.arange(896)[None, :]
    maskU = np.where(c > r + 384, NEG_BIG, 0.0).astype(np.float32)
    maskL = np.where(c < r + 384, NEG_BIG, 0.0).astype(np.float32)
    ident = np.eye(128, dtype=np.float32)
    return maskU, maskL, ident


def make_in_maps(x, W_attn, W_proj, n_cores=8, NH=8):
    N_HEAD = 16
    maskU, maskL, ident = make_masks()
    in_maps = []
    for core in range(n_cores):
        b, hg = core // 2, core % 2
        CL = NH * 64
        r0 = hg * CL
        C = x.shape[2]
        wq = np.ascontiguousarray((8.0 * W_attn[r0:r0 + CL, :]).T)
        wk = np.ascontiguousarray(W_attn[C + r0:C + r0 + CL, :].T)
        wv = np.ascontiguousarray(W_attn[2 * C + r0:2 * C + r0 + CL, :].T)
        wp = np.ascontiguousarray(W_proj[:, r0:r0 + CL].T)
        in_maps.append({
            "xT": np.ascontiguousarray(x[b].T),
            "wqT": wq, "wkT": wk, "wvT": wv, "wpT": wp,
            "maskU": maskU, "maskL": maskL, "ident": ident,
            "ones_row": np.ones((1, x.shape[1]), dtype=np.float32),
            "ones_va": np.ones((128, NH * 65), dtype=np.float32),
            "zeros_row": np.zeros((1, x.shape[1]), dtype=np.float32),
        })
    return in_maps


last_results = None


def kernel(x, W_attn, W_proj, b_proj):
    global last_results
    from concourse.bass_utils import run_bass_kernel_spmd

    x = np.asarray(x, dtype=np.float32)
    W_attn = np.asarray(W_attn, dtype=np.float32)
    W_proj = np.asarray(W_proj, dtype=np.float32)
    b_proj = np.asarray(b_proj, dtype=np.float32)

    nc = build_nc(T=2048, CK=1024, NH=8)
    in_maps = make_in_maps(x, W_attn, W_proj)
    res = run_bass_kernel_spmd(nc, in_maps, list(range(8)))
    last_results = res
    outs = []
    for b in range(4):
        o = res.results[2 * b]["outT"] + res.results[2 * b + 1]["outT"]
        outs.append(o.T + b_proj[None, :])
    return np.stack(outs).astype(np.float32)



# revision 6
# speedup vs baseline: 1.0501x; 1.0501x over previous
"""Causal self-attention Trainium2 Bass kernel.

Reference (hardcoded):
    N_EMBD=1024, N_HEAD=16, B=4, T=2048, hd=64
    qkv = x @ W_attn.T ; q,k,v split
    att = softmax(mask(q k^T * 8))          # note: reference MULTIPLIES by sqrt(hd)
    y   = (att @ v) reassembled ; out = y @ W_proj.T + b_proj

Sharding over 8 cores: core = (b, hg) with b = core//2 in 0..3 (batch),
hg = core%2 (head-group of 8 heads). Each core computes the partial c_proj
output for its 8 heads of its batch; host adds the two per-batch partials
and the bias.

Per-core layout (all matmul operands viewed as float32r):
  x    resident [128, 2048] x8 (c on partitions), loaded once
  qt/kt pair tiles [128, 2048] x4: rows 0-63 head 2a, 64-127 head 2a+1
  v    natural [t, d] in 16 tiles [128, 8*65] with a ones column per head
  pass1: S[tq,tk] blocks directly off qt/kt (K=64, base partition 64*hip)
         -> per-row max batched into nm[128,16], one PE transpose per head
  pass2: S'[tk,tq] blocks via K=65 aug matmuls (kaug row64=ones,
         qaug row64=-max) -> exp on ScalarE -> p^T in SBUF
  PV:    psum_y[65, 512] += v_aug^T.T @ p^T  (row 64 = softmax sums, free)
  norm:  per-pair: fast reciprocal of sums, selector-matmul broadcast,
         two DVE multiplies per slice
  proj:  outT[o,t] = wpT.T @ yT  accumulated over 4 c-tiles
"""

import math
from contextlib import ExitStack

import numpy as np

import concourse.bass as bass
import concourse.bacc as bacc
import concourse.mybir as mybir
import concourse.tile as tile

F32 = mybir.dt.float32
F32R = mybir.dt.float32r
AF = mybir.ActivationFunctionType
AX = mybir.AxisListType
ALU = mybir.AluOpType

NEG_BIG = -30000.0


def _R(ap):
    return ap.bitcast(F32R)


def build_nc(T=2048, CK=1024, NH=8):
    """Emit the per-core program. T: seq len, CK: embed dim (contraction),
    NH: heads on this core (head dim fixed 64)."""
    HD = 64
    NP = NH // 2          # head pairs
    CL = NH * HD          # core-local channels (512)
    NT = T // 128         # tq/tk tiles
    NS = T // 512         # 512-wide slices
    NC = CK // 128        # contraction tiles

    nc = bacc.Bacc(None, target_bir_lowering=False)

    xT = nc.declare_dram_parameter("xT", [CK, T], F32, isOutput=False)
    wqT = nc.declare_dram_parameter("wqT", [CK, CL], F32, isOutput=False)
    wkT = nc.declare_dram_parameter("wkT", [CK, CL], F32, isOutput=False)
    wvT = nc.declare_dram_parameter("wvT", [CK, CL], F32, isOutput=False)
    wpT = nc.declare_dram_parameter("wpT", [CL, CK], F32, isOutput=False)
    maskU = nc.declare_dram_parameter("maskU", [128, 896], F32, isOutput=False)
    maskL = nc.declare_dram_parameter("maskL", [128, 896], F32, isOutput=False)
    ident = nc.declare_dram_parameter("ident", [128, 128], F32, isOutput=False)
    ones_row = nc.declare_dram_parameter("ones_row", [1, T], F32, isOutput=False)
    ones_va = nc.declare_dram_parameter("ones_va", [128, NH * 65], F32, isOutput=False)
    sel_all = nc.declare_dram_parameter("sel_all", [8, 512], F32, isOutput=False)
    outT = nc.declare_dram_parameter("outT", [CK, T], F32, isOutput=True)

    with tile.TileContext(nc) as tc, ExitStack() as ctx:
        singles = ctx.enter_context(tc.tile_pool(name="singles", bufs=1))

        ident_sb = singles.tile([128, 128], F32, tag="ident")
        nc.sync.dma_start(out=ident_sb, in_=ident[:, :])
        identR_sb = singles.tile([128, 128], F32, tag="identR")
        nc.sync.dma_start(out=_R(identR_sb), in_=_R(ident[:, :]))
        maskU_sb = singles.tile([128, 896], F32, tag="maskU")
        nc.sync.dma_start(out=_R(maskU_sb), in_=_R(maskU[:, :]))
        maskL_sb = singles.tile([128, 896], F32, tag="maskL")
        nc.sync.dma_start(out=_R(maskL_sb), in_=_R(maskL[:, :]))
        sel_sb = singles.tile([8, 512], F32, tag="sel")
        nc.sync.dma_start(out=_R(sel_sb), in_=_R(sel_all[:, :]))

        # resident activations
        qt = [singles.tile([128, T], F32, tag=f"qt{a}", name=f"qt{a}") for a in range(NP)]
        kt = [singles.tile([128, T], F32, tag=f"kt{a}", name=f"kt{a}") for a in range(NP)]
        va = [singles.tile([128, NH * 65], F32, tag=f"va{i}", name=f"va{i}") for i in range(NT)]

        for i in range(NT):
            nc.sync.dma_start(out=_R(va[i][:, :]), in_=_R(ones_va[:, :]))

        # ---------------- phase P: projections ----------------
        # x resident across both sub-phases (loaded once), freed before A
        with tc.tile_pool(name="xpool", bufs=1) as xpool, \
             tc.tile_pool(name="ppj", bufs=4, space="PSUM") as ppj:
            xsb = [xpool.tile([128, T], F32, tag=f"x{c}", name=f"x{c}")
                   for c in range(NC)]
            for c in range(NC):
                nc.sync.dma_start(out=_R(xsb[c]), in_=_R(xT[128 * c:128 * (c + 1), :]))
            with tc.tile_pool(name="wpool1", bufs=1) as wpool:
                wq_sb = [wpool.tile([128, CL], F32, tag=f"wq{c}", name=f"wq{c}")
                         for c in range(NC)]
                wk_sb = [wpool.tile([128, CL], F32, tag=f"wk{c}", name=f"wk{c}")
                         for c in range(NC)]
                for c in range(NC):
                    nc.sync.dma_start(out=_R(wq_sb[c]), in_=_R(wqT[128 * c:128 * (c + 1), :]))
                    nc.sync.dma_start(out=_R(wk_sb[c]), in_=_R(wkT[128 * c:128 * (c + 1), :]))
                for a in range(NP):
                    for s in range(NS):
                        for w_sb, dest in ((wq_sb, qt), (wk_sb, kt)):
                            ps = ppj.tile([128, 512], F32, tag="pj", name="pj_qk")
                            for c in range(NC):
                                nc.tensor.matmul(
                                    ps, _R(w_sb[c][:, 128 * a:128 * (a + 1)]),
                                    _R(xsb[c][:, 512 * s:512 * (s + 1)]),
                                    start=(c == 0), stop=(c == NC - 1))
                            nc.scalar.copy(_R(dest[a][:, 512 * s:512 * (s + 1)]), ps)

            with tc.tile_pool(name="wpool2", bufs=1) as wpool:
                wv_sb = [wpool.tile([128, CL], F32, tag=f"wv{c}", name=f"wv{c}")
                         for c in range(NC)]
                for c in range(NC):
                    nc.sync.dma_start(out=_R(wv_sb[c]), in_=_R(wvT[128 * c:128 * (c + 1), :]))
                for i in range(NT):
                    ps = ppj.tile([128, CL], F32, tag="pj", name="pj_v")
                    for c in range(NC):
                        nc.tensor.matmul(
                            ps, _R(xsb[c][:, 128 * i:128 * (i + 1)]), _R(wv_sb[c]),
                            start=(c == 0), stop=(c == NC - 1))
                    va_view = va[i].rearrange("p (h e) -> p h e", e=65)
                    ps_view = ps.rearrange("p (h e) -> p h e", e=HD)
                    nc.vector.tensor_copy(_R(va_view[:, :, 0:HD]), ps_view)

        # ---------------- phase A: attention ----------------
        ytpool = ctx.enter_context(tc.tile_pool(name="ytpool", bufs=1))
        yt = [ytpool.tile([128, T], F32, tag=f"yt{a}", name=f"yt{a}") for a in range(NP)]
        su8s = [None] * NP
        rinv8s = [None] * NP
        rinv8rs = [None] * NP
        with tc.tile_pool(name="p1ps", bufs=3, space="PSUM") as p1ps, \
             tc.tile_pool(name="tpp", bufs=1, space="PSUM") as tpp, \
             tc.tile_pool(name="p2ps", bufs=2, space="PSUM") as p2ps, \
             tc.tile_pool(name="yvps", bufs=2, space="PSUM") as yvps, \
             tc.tile_pool(name="augs", bufs=2) as augs, \
             tc.tile_pool(name="ptp", bufs=3) as ptp, \
             tc.tile_pool(name="small", bufs=2) as small:

            def head_setup(h):
                """Per-head pass-2 tiles: kaug row64 = ones; qaug row64 is
                filled with -max by p1_done's DMA."""
                a, hip = h // 2, h % 2
                kaug = augs.tile([65, T], F32, tag="kaug", name=f"kaug{h}")
                nc.sync.dma_start(out=_R(kaug[0:64, :]),
                                  in_=_R(kt[a][64 * hip:64 * hip + 64, :]))
                nc.sync.dma_start(out=_R(kaug[64:65, :]), in_=_R(ones_row[:, :]))
                qaug = augs.tile([65, T], F32, tag="qaug", name=f"qaug{h}")
                nc.sync.dma_start(out=_R(qaug[0:64, :]),
                                  in_=_R(qt[a][64 * hip:64 * hip + 64, :]))
                nm = small.tile([128, 16], F32, tag="nm", name=f"nm{h}")
                tps16 = small.tile([16, 128], F32, tag="tps16", name=f"tps16{h}")
                if hip == 0:
                    su8s[a] = small.tile([8, 512], F32, tag="su8",
                                         name=f"su8_{a}")
                    rinv8s[a] = small.tile([8, 512], F32, tag="rinv8",
                                           name=f"rinv8_{a}")
                    rinv8rs[a] = small.tile([8, 512], F32, tag="rinv8r",
                                            name=f"rinv8r_{a}")
                ytmp = [small.tile([65, 512], F32, tag=f"ytmp{hip}_{s}",
                                   name=f"ytmp{h}_{s}", bufs=1) for s in range(NS)]
                return dict(h=h, kaug=kaug, qaug=qaug, nm=nm, tps16=tps16,
                            ytmp=ytmp)

            def p1_row(st, i):
                """Pass-1 row i for head h: causal-trimmed S blocks via K=64
                matmuls straight off qt/kt; -max of each row into nm[:, i]."""
                h, nm = st["h"], st["nm"]
                a, hip = h // 2, h % 2
                r0 = 64 * hip
                jd, m = i // 4, i % 4
                W = 128 * (m + 1)            # diag block valid width
                mA = small.tile([128, 4], F32, tag="mA", name="mA")
                for j in range(jd + 1):
                    diag = (j == jd)
                    w = W if diag else 512
                    psA = p1ps.tile([128, 512], F32, tag="blk", name="psA")
                    nc.tensor.matmul(
                        psA[:, 0:w], _R(qt[a][r0:r0 + 64, 128 * i:128 * (i + 1)]),
                        _R(kt[a][r0:r0 + 64, 512 * j:512 * j + w]),
                        start=True, stop=not diag)
                    if diag:
                        # boundary 128-chunk gets the pure triangular mask
                        msl = _R(maskU_sb[:, 384:512])
                        nc.tensor.matmul(psA[:, W - 128:W], _R(identR_sb), msl,
                                         start=False, stop=True)
                    nc.vector.reduce_max(mA[:, j:j + 1], psA[:, 0:w], axis=AX.X)
                nc.vector.tensor_reduce(
                    nm[:, i:i + 1], mA[:, 0:jd + 1], axis=AX.X, op=ALU.max,
                    negate=True)

            def p1_done(st):
                # batched transpose of the 16 per-row negmax columns, then one
                # DMA drops them into qaug row 64 as the pass-2 bias row
                h, nm, tps16 = st["h"], st["nm"], st["tps16"]
                tp = tpp.tile([16, 128], F32, tag="tp", name="tp")
                nc.tensor.transpose(tp, nm, ident_sb)
                nc.vector.tensor_copy(tps16, tp)
                nc.sync.dma_start(out=_R(st["qaug"][64:65, :]), in_=_R(tps16))

            def a2_slice(st, s):
                """Pass-2 + PV for (head, tq-slice s), causal-trimmed."""
                h, kaug, qaug = st["h"], st["kaug"], st["qaug"]
                hip = h % 2
                nts = 4 * s + 4
                yps = yvps.tile([65, 512], F32, tag="y", name="yps")
                for t in range(nts):
                    mp = t - 4 * s
                    c0 = 128 * mp if mp > 0 else 0   # first valid column
                    ps2 = p2ps.tile([128, 512], F32, tag="s2", name="ps2")
                    nc.tensor.matmul(
                        ps2[:, c0:512], _R(kaug[:, 128 * t:128 * (t + 1)]),
                        _R(qaug[:, 512 * s + c0:512 * (s + 1)]),
                        start=True, stop=(mp < 0))
                    if mp >= 0:
                        msl = _R(maskL_sb[:, 384:512])
                        nc.tensor.matmul(ps2[:, c0:c0 + 128], _R(identR_sb), msl,
                                         start=False, stop=True)
                    pt = ptp.tile([128, 512], F32, tag="pt", name="pt")
                    nc.scalar.activation(_R(pt[:, c0:512]), ps2[:, c0:512], AF.Exp)
                    nc.tensor.matmul(
                        yps[:, c0:512], _R(va[t][:, 65 * h:65 * h + 65]),
                        _R(pt[:, c0:512]),
                        start=(t == 0), stop=(t == nts - 1),
                        skip_group_check=True)
                nc.scalar.copy(st["ytmp"][s], yps)
                a = h // 2
                nc.sync.dma_start(out=su8s[a][4 * hip + s:4 * hip + s + 1, :],
                                  in_=st["ytmp"][s][64:65, :])

            def a2_recip(st):
                h = st["h"]
                a, hip = h // 2, h % 2
                if hip == 1:
                    nc.vector.reciprocal_approx_fast(rinv8s[a], su8s[a])
                    nc.vector.tensor_copy(_R(rinv8rs[a]), rinv8s[a])

            def pair_finish(st_lo, st_hi, s):
                """Scale both heads' slice s by 1/sum and write into yt."""
                h = st_hi["h"]
                a = h // 2
                sc = yvps.tile([128, 512], F32, tag="y", name="sc")
                nc.tensor.matmul(sc, _R(sel_sb[:, 128 * s:128 * (s + 1)]),
                                 _R(rinv8rs[a]), start=True, stop=True)
                nc.vector.tensor_mul(
                    _R(yt[a][0:64, 512 * s:512 * (s + 1)]),
                    st_lo["ytmp"][s][0:64, :], sc[0:64, :])
                nc.vector.tensor_mul(
                    _R(yt[a][64:128, 512 * s:512 * (s + 1)]),
                    st_hi["ytmp"][s][0:64, :], sc[64:128, :])

            # software pipeline at head granularity: pass-1 of head h runs
            # (PE-light, DVE-heavy) interleaved with pass-2/PV of head h-1
            # (PE-heavy) so both engines stay fed.
            def a2_sections(st):
                for s in range(NS):
                    yield lambda st=st, s=s: a2_slice(st, s)
                yield lambda st=st: a2_recip(st)
                if st["h"] % 2 == 1:
                    lo = sts[st["h"] - 1]
                    for s in range(NS):
                        yield lambda lo=lo, st=st, s=s: pair_finish(lo, st, s)

            sts = [None] * NH
            for h in range(NH + 1):
                prev_iter = a2_sections(sts[h - 1]) if h >= 1 else iter(())
                if h < NH:
                    sts[h] = head_setup(h)
                    for i in range(NT):
                        p1_row(sts[h], i)
                        if i % 3 == 2:
                            nxt = next(prev_iter, None)
                            if nxt is not None:
                                nxt()
                    p1_done(sts[h])
                for nxt in prev_iter:
                    nxt()

        # ---------------- phase PR: output projection ----------------
        with tc.tile_pool(name="wppool", bufs=1) as wppool, \
             tc.tile_pool(name="prps", bufs=4, space="PSUM") as prps, \
             tc.tile_pool(name="stg", bufs=3) as stg:
            wp_sb = [wppool.tile([128, CK], F32, tag=f"wp{t_}", name=f"wp{t_}")
                     for t_ in range(NP)]
            for t_ in range(NP):
                nc.sync.dma_start(out=_R(wp_sb[t_]), in_=_R(wpT[128 * t_:128 * (t_ + 1), :]))
            for o in range(NC):
                for s in range(NS):
                    ps = prps.tile([128, 512], F32, tag="pr")
                    for t_ in range(NP):
                        nc.tensor.matmul(
                            ps, _R(wp_sb[t_][:, 128 * o:128 * (o + 1)]),
                            _R(yt[t_][:, 512 * s:512 * (s + 1)]),
                            start=(t_ == 0), stop=(t_ == NP - 1))
                    st = stg.tile([128, 512], F32, tag="st")
                    nc.scalar.copy(st, ps)
                    nc.sync.dma_start(
                        out=outT[128 * o:128 * (o + 1), 512 * s:512 * (s + 1)], in_=st)

    nc.finalize()
    return nc


def make_masks():
    r = np.arange(128)[:, None]
    c = np.arange(896)[None, :]
    maskU = np.where(c > r + 384, NEG_BIG, 0.0).astype(np.float32)
    maskL = np.where(c < r + 384, NEG_BIG, 0.0).astype(np.float32)
    ident = np.eye(128, dtype=np.float32)
    return maskU, maskL, ident


def make_sel():
    # sel_all[r, 128*s + p]: block s row (4*(p>=64) + s) is one
    sel = np.zeros((8, 512), dtype=np.float32)
    for s in range(4):
        sel[s, 128 * s:128 * s + 64] = 1.0
        sel[4 + s, 128 * s + 64:128 * (s + 1)] = 1.0
    return sel


def make_in_maps(x, W_attn, W_proj, n_cores=8, NH=8):
    maskU, maskL, ident = make_masks()
    sel = make_sel()
    in_maps = []
    for core in range(n_cores):
        b, hg = core // 2, core % 2
        CL = NH * 64
        r0 = hg * CL
        C = x.shape[2]
        wq = np.ascontiguousarray((8.0 * W_attn[r0:r0 + CL, :]).T)
        wk = np.ascontiguousarray(W_attn[C + r0:C + r0 + CL, :].T)
        wv = np.ascontiguousarray(W_attn[2 * C + r0:2 * C + r0 + CL, :].T)
        wp = np.ascontiguousarray(W_proj[:, r0:r0 + CL].T)
        in_maps.append({
            "xT": np.ascontiguousarray(x[b].T),
            "wqT": wq, "wkT": wk, "wvT": wv, "wpT": wp,
            "maskU": maskU, "maskL": maskL, "ident": ident,
            "ones_row": np.ones((1, x.shape[1]), dtype=np.float32),
            "ones_va": np.ones((128, NH * 65), dtype=np.float32),
            "sel_all": sel,
        })
    return in_maps


last_results = None


def kernel(x, W_attn, W_proj, b_proj):
    global last_results
    from concourse.bass_utils import run_bass_kernel_spmd

    x = np.asarray(x, dtype=np.float32)
    W_attn = np.asarray(W_attn, dtype=np.float32)
    W_proj = np.asarray(W_proj, dtype=np.float32)
    b_proj = np.asarray(b_proj, dtype=np.float32)

    nc = build_nc(T=2048, CK=1024, NH=8)
    in_maps = make_in_maps(x, W_attn, W_proj)
    res = run_bass_kernel_spmd(nc, in_maps, list(range(8)))
    last_results = res
    outs = []
    for b in range(4):
        o = res.results[2 * b]["outT"] + res.results[2 * b + 1]["outT"]
        outs.append(o.T + b_proj[None, :])
    return np.stack(outs).astype(np.float32)


# revision 8
# speedup vs baseline: 1.3811x; 1.3151x over previous
"""Causal self-attention Trainium2 Bass kernel.

Reference (hardcoded):
    N_EMBD=1024, N_HEAD=16, B=4, T=2048, hd=64
    qkv = x @ W_attn.T ; q,k,v split
    att = softmax(mask(q k^T * 8))          # note: reference MULTIPLIES by sqrt(hd)
    y   = (att @ v) reassembled ; out = y @ W_proj.T + b_proj

Sharding over 8 cores: core = (b, hg) with b = core//2 in 0..3 (batch),
hg = core%2 (head-group of 8 heads). Each core computes the partial c_proj
output for its 8 heads of its batch; host adds the two per-batch partials
and the bias.

Per-core layout (all matmul operands viewed as float32r):
  x    resident [128, 2048] x8 (c on partitions), loaded once
  qt/kt pair tiles [128, 2048] x4: rows 0-63 head 2a, 64-127 head 2a+1
  v    natural [t, d] in 16 tiles [128, 8*65] with a ones column per head
  pass1: S[tq,tk] blocks directly off qt/kt (K=64, base partition 64*hip)
         -> per-row max batched into nm[128,16], one PE transpose per head
  pass2: S'[tk,tq] blocks via K=65 aug matmuls (kaug row64=ones,
         qaug row64=-max) -> exp on ScalarE -> p^T in SBUF
  PV:    psum_y[65, 512] += v_aug^T.T @ p^T  (row 64 = softmax sums, free)
  norm:  per-pair: fast reciprocal of sums, selector-matmul broadcast,
         two DVE multiplies per slice
  proj:  outT[o,t] = wpT.T @ yT  accumulated over 4 c-tiles
"""

import math
from contextlib import ExitStack

import numpy as np

import concourse.bass as bass
import concourse.bacc as bacc
import concourse.mybir as mybir
import concourse.tile as tile

F32 = mybir.dt.float32
F32R = mybir.dt.float32r
BF16 = mybir.dt.bfloat16
AF = mybir.ActivationFunctionType
AX = mybir.AxisListType
ALU = mybir.AluOpType

NEG_BIG = -30000.0


def _R(ap):
    return ap.bitcast(F32R)


def build_nc(T=2048, CK=1024, NH=8):
    """Emit the per-core program. T: seq len, CK: embed dim (contraction),
    NH: heads on this core (head dim fixed 64)."""
    HD = 64
    NP = NH // 2          # head pairs
    CL = NH * HD          # core-local channels (512)
    NT = T // 128         # tq/tk tiles
    NS = T // 512         # 512-wide slices
    NC = CK // 128        # contraction tiles

    nc = bacc.Bacc(None, target_bir_lowering=False)

    xT = nc.declare_dram_parameter("xT", [CK, T], F32, isOutput=False)
    wqT = nc.declare_dram_parameter("wqT", [CK, CL], F32, isOutput=False)
    wkT = nc.declare_dram_parameter("wkT", [CK, CL], F32, isOutput=False)
    wvT = nc.declare_dram_parameter("wvT", [CK, CL], F32, isOutput=False)
    wpT = nc.declare_dram_parameter("wpT", [CL, CK], F32, isOutput=False)
    maskUb = nc.declare_dram_parameter("maskUb", [128, 128], BF16, isOutput=False)
    maskLb = nc.declare_dram_parameter("maskLb", [128, 128], BF16, isOutput=False)
    ident = nc.declare_dram_parameter("ident", [128, 128], F32, isOutput=False)
    identb = nc.declare_dram_parameter("identb", [128, 128], BF16, isOutput=False)
    ones_row = nc.declare_dram_parameter("ones_row", [1, T], F32, isOutput=False)
    ones_va = nc.declare_dram_parameter("ones_va", [128, NH * 128], BF16, isOutput=False)
    sel_all = nc.declare_dram_parameter("sel_all", [128, 512], F32, isOutput=False)
    zeros_pad = nc.declare_dram_parameter("zeros_pad", [120, T], F32, isOutput=False)
    outT = nc.declare_dram_parameter("outT", [CK, T], F32, isOutput=True)

    with tile.TileContext(nc) as tc, ExitStack() as ctx:
        singles = ctx.enter_context(tc.tile_pool(name="singles", bufs=1))

        ident_sb = singles.tile([128, 128], F32, tag="ident")
        nc.sync.dma_start(out=ident_sb, in_=ident[:, :])
        identR_sb = singles.tile([128, 128], F32, tag="identR")
        nc.sync.dma_start(out=_R(identR_sb), in_=_R(ident[:, :]))
        maskU_sb = singles.tile([128, 128], BF16, tag="maskU")
        nc.sync.dma_start(out=maskU_sb, in_=maskUb[:, :])
        maskL_sb = singles.tile([128, 128], BF16, tag="maskL")
        nc.sync.dma_start(out=maskL_sb, in_=maskLb[:, :])
        identb_sb = singles.tile([128, 128], BF16, tag="identb")
        nc.sync.dma_start(out=identb_sb, in_=identb[:, :])
        sel_sb = singles.tile([128, 512], F32, tag="sel")
        nc.sync.dma_start(out=_R(sel_sb), in_=_R(sel_all[:, :]))

        # resident activations
        qt = [singles.tile([128, T], F32, tag=f"qt{a}", name=f"qt{a}") for a in range(NP)]
        kt = [singles.tile([128, T], F32, tag=f"kt{a}", name=f"kt{a}") for a in range(NP)]
        va = [singles.tile([128, NH * 128], BF16, tag=f"va{i}", name=f"va{i}") for i in range(NT)]

        for i in range(NT):
            nc.sync.dma_start(out=va[i][:, :], in_=ones_va[:, :])

        # ---------------- phase P: projections ----------------
        # x resident across both sub-phases (loaded once), freed before A
        with tc.tile_pool(name="xpool", bufs=1) as xpool, \
             tc.tile_pool(name="ppj", bufs=4, space="PSUM") as ppj:
            xsb = [xpool.tile([128, T], F32, tag=f"x{c}", name=f"x{c}")
                   for c in range(NC)]
            for c in range(NC):
                nc.sync.dma_start(out=_R(xsb[c]), in_=_R(xT[128 * c:128 * (c + 1), :]))
            with tc.tile_pool(name="wpool1", bufs=1) as wpool:
                wq_sb = [wpool.tile([128, CL], F32, tag=f"wq{c}", name=f"wq{c}")
                         for c in range(NC)]
                wk_sb = [wpool.tile([128, CL], F32, tag=f"wk{c}", name=f"wk{c}")
                         for c in range(NC)]
                for c in range(NC):
                    nc.sync.dma_start(out=_R(wq_sb[c]), in_=_R(wqT[128 * c:128 * (c + 1), :]))
                    nc.sync.dma_start(out=_R(wk_sb[c]), in_=_R(wkT[128 * c:128 * (c + 1), :]))
                for a in range(NP):
                    for s in range(NS):
                        for w_sb, dest in ((wq_sb, qt), (wk_sb, kt)):
                            ps = ppj.tile([128, 512], F32, tag="pj", name="pj_qk")
                            for c in range(NC):
                                nc.tensor.matmul(
                                    ps, _R(w_sb[c][:, 128 * a:128 * (a + 1)]),
                                    _R(xsb[c][:, 512 * s:512 * (s + 1)]),
                                    start=(c == 0), stop=(c == NC - 1))
                            nc.scalar.copy(_R(dest[a][:, 512 * s:512 * (s + 1)]), ps)

            with tc.tile_pool(name="wpool2", bufs=1) as wpool:
                wv_sb = [wpool.tile([128, CL], F32, tag=f"wv{c}", name=f"wv{c}")
                         for c in range(NC)]
                for c in range(NC):
                    nc.sync.dma_start(out=_R(wv_sb[c]), in_=_R(wvT[128 * c:128 * (c + 1), :]))
                for i in range(NT):
                    ps = ppj.tile([128, CL], F32, tag="pj", name="pj_v")
                    for c in range(NC):
                        nc.tensor.matmul(
                            ps, _R(xsb[c][:, 128 * i:128 * (i + 1)]), _R(wv_sb[c]),
                            start=(c == 0), stop=(c == NC - 1))
                    va_view = va[i].rearrange("p (h e) -> p h e", e=128)
                    ps_view = ps.rearrange("p (h e) -> p h e", e=HD)
                    nc.vector.tensor_copy(va_view[:, :, 0:HD], ps_view)

        # ---------------- phase A: attention ----------------
        ytpool = ctx.enter_context(tc.tile_pool(name="ytpool", bufs=1))
        yt = [ytpool.tile([128, T], F32, tag=f"yt{a}", name=f"yt{a}") for a in range(NP)]
        su8s = [None] * NP
        rinv8s = [None] * NP
        rinv8rs = [None] * NP
        with tc.tile_pool(name="p1ps", bufs=3, space="PSUM") as p1ps, \
             tc.tile_pool(name="tpp", bufs=1, space="PSUM") as tpp, \
             tc.tile_pool(name="p2ps", bufs=2, space="PSUM") as p2ps, \
             tc.tile_pool(name="yvps", bufs=2, space="PSUM") as yvps, \
             tc.tile_pool(name="augs", bufs=2) as augs, \
             tc.tile_pool(name="ptp", bufs=3) as ptp, \
             tc.tile_pool(name="small", bufs=2) as small:

            def head_setup(h):
                """Per-head pass-2 tiles: kaug row64 = ones; qaug row64 is
                filled with -max by p1_done's DMA."""
                a, hip = h // 2, h % 2
                kaug = augs.tile([128, T], F32, tag="kaug", name=f"kaug{h}")
                nc.sync.dma_start(out=_R(kaug[0:64, :]),
                                  in_=_R(kt[a][64 * hip:64 * hip + 64, :]))
                nc.sync.dma_start(out=_R(kaug[64:65, :]), in_=_R(ones_row[:, :]))
                nc.sync.dma_start(out=_R(kaug[65:128, :]), in_=_R(zeros_pad[0:63, :]))
                qaug = augs.tile([128, T], F32, tag="qaug", name=f"qaug{h}")
                nc.sync.dma_start(out=_R(qaug[0:64, :]),
                                  in_=_R(qt[a][64 * hip:64 * hip + 64, :]))
                nc.sync.dma_start(out=_R(qaug[64:128, :]), in_=_R(zeros_pad[0:64, :]))
                nm = small.tile([128, 16], F32, tag="nm", name=f"nm{h}")
                tps16 = small.tile([16, 128], F32, tag="tps16", name=f"tps16{h}")
                if hip == 0:
                    su8s[a] = small.tile([8, 512], F32, tag="su8",
                                         name=f"su8_{a}")
                    rinv8s[a] = small.tile([8, 512], F32, tag="rinv8",
                                           name=f"rinv8_{a}")
                    rinv8rs[a] = small.tile([128, 512], F32, tag="rinv8r",
                                            name=f"rinv8r_{a}")
                    nc.sync.dma_start(out=_R(rinv8rs[a][8:128, :]),
                                      in_=_R(zeros_pad[0:120, 0:512]))
                ytmp = [small.tile([65, 512], F32, tag=f"ytmp{hip}_{s}",
                                   name=f"ytmp{h}_{s}", bufs=1) for s in range(NS)]
                return dict(h=h, kaug=kaug, qaug=qaug, nm=nm, tps16=tps16,
                            ytmp=ytmp)

            def p1_row(st, i):
                """Pass-1 row i for head h: causal-trimmed S blocks off the
                padded aug tiles (stationary full [128,128]; qaug row 64 is
                still zero here so kaug's ones row contributes nothing);
                -max of each row into nm[:, i]."""
                h, nm = st["h"], st["nm"]
                kaug, qaug = st["kaug"], st["qaug"]
                jd, m = i // 4, i % 4
                W = 128 * (m + 1)            # diag block valid width
                mA = small.tile([128, 4], F32, tag="mA", name="mA")
                for j in range(jd + 1):
                    diag = (j == jd)
                    w = W if diag else 512
                    psA = p1ps.tile([128, 512], F32, tag="blk", name="psA")
                    nc.tensor.matmul(
                        psA[:, 0:w], _R(qaug[:, 128 * i:128 * (i + 1)]),
                        _R(kaug[:, 512 * j:512 * j + w]),
                        start=True, stop=not diag)
                    if diag:
                        # boundary 128-chunk gets the pure triangular mask
                        nc.tensor.matmul(psA[:, W - 128:W], identb_sb,
                                         maskU_sb, start=False, stop=True)
                    nc.vector.reduce_max(mA[:, j:j + 1], psA[:, 0:w], axis=AX.X)
                nc.vector.tensor_reduce(
                    nm[:, i:i + 1], mA[:, 0:jd + 1], axis=AX.X, op=ALU.max,
                    negate=True)

            def p1_done(st):
                # batched transpose of the 16 per-row negmax columns, then one
                # DMA drops them into qaug row 64 as the pass-2 bias row
                h, nm, tps16 = st["h"], st["nm"], st["tps16"]
                tp = tpp.tile([16, 128], F32, tag="tp", name="tp")
                nc.tensor.transpose(tp, nm, ident_sb)
                nc.vector.tensor_copy(tps16, tp)
                nc.sync.dma_start(out=_R(st["qaug"][64:65, :]), in_=_R(tps16))

            def a2_slice(st, s):
                """Pass-2 + PV for (head, tq-slice s), causal-trimmed."""
                h, kaug, qaug = st["h"], st["kaug"], st["qaug"]
                hip = h % 2
                nts = 4 * s + 4
                yps = yvps.tile([128, 512], F32, tag="y", name="yps")
                for t in range(nts):
                    mp = t - 4 * s
                    c0 = 128 * mp if mp > 0 else 0   # first valid column
                    ps2 = p2ps.tile([128, 512], F32, tag="s2", name="ps2")
                    nc.tensor.matmul(
                        ps2[:, c0:512], _R(kaug[:, 128 * t:128 * (t + 1)]),
                        _R(qaug[:, 512 * s + c0:512 * (s + 1)]),
                        start=True, stop=(mp < 0))
                    if mp >= 0:
                        nc.tensor.matmul(ps2[:, c0:c0 + 128], identb_sb,
                                         maskL_sb, start=False, stop=True)
                    pt = ptp.tile([128, 512], BF16, tag="pt", name="pt")
                    nc.scalar.activation(pt[:, c0:512], ps2[:, c0:512], AF.Exp)
                    nc.tensor.matmul(
                        yps[:, c0:512], va[t][:, 128 * h:128 * h + 128],
                        pt[:, c0:512],
                        start=(t == 0), stop=(t == nts - 1),
                        skip_group_check=True)
                nc.scalar.copy(st["ytmp"][s], yps[0:65, :])
                a = h // 2
                nc.sync.dma_start(out=su8s[a][4 * hip + s:4 * hip + s + 1, :],
                                  in_=st["ytmp"][s][64:65, :])

            def a2_recip(st):
                h = st["h"]
                a, hip = h // 2, h % 2
                if hip == 1:
                    nc.vector.reciprocal_approx_fast(rinv8s[a], su8s[a])
                    nc.vector.tensor_copy(_R(rinv8rs[a][0:8, :]), rinv8s[a])

            def pair_finish(st_lo, st_hi, s):
                """Scale both heads' slice s by 1/sum and write into yt."""
                h = st_hi["h"]
                a = h // 2
                sc = yvps.tile([128, 512], F32, tag="y", name="sc")
                nc.tensor.matmul(sc, _R(sel_sb[:, 128 * s:128 * (s + 1)]),
                                 _R(rinv8rs[a]), start=True, stop=True)
                nc.vector.tensor_mul(
                    _R(yt[a][0:64, 512 * s:512 * (s + 1)]),
                    st_lo["ytmp"][s][0:64, :], sc[0:64, :])
                nc.vector.tensor_mul(
                    _R(yt[a][64:128, 512 * s:512 * (s + 1)]),
                    st_hi["ytmp"][s][0:64, :], sc[64:128, :])

            # software pipeline at head granularity: pass-1 of head h runs
            # (PE-light, DVE-heavy) interleaved with pass-2/PV of head h-1
            # (PE-heavy) so both engines stay fed.
            def a2_sections(st):
                for s in range(NS):
                    yield lambda st=st, s=s: a2_slice(st, s)
                yield lambda st=st: a2_recip(st)
                if st["h"] % 2 == 1:
                    lo = sts[st["h"] - 1]
                    for s in range(NS):
                        yield lambda lo=lo, st=st, s=s: pair_finish(lo, st, s)

            sts = [None] * NH
            for h in range(NH + 1):
                prev_iter = a2_sections(sts[h - 1]) if h >= 1 else iter(())
                if h < NH:
                    sts[h] = head_setup(h)
                    for i in range(NT):
                        p1_row(sts[h], i)
                        if i % 3 == 2:
                            nxt = next(prev_iter, None)
                            if nxt is not None:
                                nxt()
                    p1_done(sts[h])
                for nxt in prev_iter:
                    nxt()

        # ---------------- phase PR: output projection ----------------
        with tc.tile_pool(name="wppool", bufs=1) as wppool, \
             tc.tile_pool(name="prps", bufs=4, space="PSUM") as prps, \
             tc.tile_pool(name="stg", bufs=3) as stg:
            wp_sb = [wppool.tile([128, CK], F32, tag=f"wp{t_}", name=f"wp{t_}")
                     for t_ in range(NP)]
            for t_ in range(NP):
                nc.sync.dma_start(out=_R(wp_sb[t_]), in_=_R(wpT[128 * t_:128 * (t_ + 1), :]))
            for o in range(NC):
                for s in range(NS):
                    ps = prps.tile([128, 512], F32, tag="pr")
                    for t_ in range(NP):
                        nc.tensor.matmul(
                            ps, _R(wp_sb[t_][:, 128 * o:128 * (o + 1)]),
                            _R(yt[t_][:, 512 * s:512 * (s + 1)]),
                            start=(t_ == 0), stop=(t_ == NP - 1))
                    st = stg.tile([128, 512], F32, tag="st")
                    nc.scalar.copy(st, ps)
                    nc.sync.dma_start(
                        out=outT[128 * o:128 * (o + 1), 512 * s:512 * (s + 1)], in_=st)

    nc.finalize()
    return nc


def make_masks():
    from ml_dtypes import bfloat16
    r = np.arange(128)[:, None]
    c = np.arange(128)[None, :]
    maskU = np.where(c > r, NEG_BIG, 0.0).astype(bfloat16)
    maskL = np.where(c < r, NEG_BIG, 0.0).astype(bfloat16)
    ident = np.eye(128, dtype=np.float32)
    identb = np.eye(128, dtype=np.float32).astype(bfloat16)
    return maskU, maskL, ident, identb


def make_sel():
    # sel_all[r, 128*s + p]: block s row (4*(p>=64) + s) is one
    sel = np.zeros((128, 512), dtype=np.float32)
    for s in range(4):
        sel[s, 128 * s:128 * s + 64] = 1.0
        sel[4 + s, 128 * s + 64:128 * (s + 1)] = 1.0
    return sel


def make_in_maps(x, W_attn, W_proj, n_cores=8, NH=8):
    from ml_dtypes import bfloat16
    maskU, maskL, ident, identb = make_masks()
    sel = make_sel()
    T = x.shape[1]
    ones_va = np.zeros((128, NH * 128), dtype=bfloat16)
    ones_va[:, 64::128] = 1.0
    in_maps = []
    for core in range(n_cores):
        b, hg = core // 2, core % 2
        CL = NH * 64
        r0 = hg * CL
        C = x.shape[2]
        wq = np.ascontiguousarray((8.0 * W_attn[r0:r0 + CL, :]).T)
        wk = np.ascontiguousarray(W_attn[C + r0:C + r0 + CL, :].T)
        wv = np.ascontiguousarray(W_attn[2 * C + r0:2 * C + r0 + CL, :].T)
        wp = np.ascontiguousarray(W_proj[:, r0:r0 + CL].T)
        in_maps.append({
            "xT": np.ascontiguousarray(x[b].T),
            "wqT": wq, "wkT": wk, "wvT": wv, "wpT": wp,
            "maskUb": maskU, "maskLb": maskL, "ident": ident,
            "identb": identb,
            "ones_row": np.ones((1, T), dtype=np.float32),
            "ones_va": ones_va,
            "sel_all": sel,
            "zeros_pad": np.zeros((120, T), dtype=np.float32),
        })
    return in_maps


last_results = None


def kernel(x, W_attn, W_proj, b_proj):
    global last_results
    from concourse.bass_utils import run_bass_kernel_spmd

    x = np.asarray(x, dtype=np.float32)
    W_attn = np.asarray(W_attn, dtype=np.float32)
    W_proj = np.asarray(W_proj, dtype=np.float32)
    b_proj = np.asarray(b_proj, dtype=np.float32)

    nc = build_nc(T=2048, CK=1024, NH=8)
    in_maps = make_in_maps(x, W_attn, W_proj)
    res = run_bass_kernel_spmd(nc, in_maps, list(range(8)))
    last_results = res
    outs = []
    for b in range(4):
        o = res.results[2 * b]["outT"] + res.results[2 * b + 1]["outT"]
        outs.append(o.T + b_proj[None, :])
    return np.stack(outs).astype(np.float32)


# revision 9
# speedup vs baseline: 1.3937x; 1.0091x over previous
"""Causal self-attention Trainium2 Bass kernel.

Reference (hardcoded):
    N_EMBD=1024, N_HEAD=16, B=4, T=2048, hd=64
    qkv = x @ W_attn.T ; q,k,v split
    att = softmax(mask(q k^T * 8))          # note: reference MULTIPLIES by sqrt(hd)
    y   = (att @ v) reassembled ; out = y @ W_proj.T + b_proj

Sharding over 8 cores: core = (b, hg) with b = core//2 in 0..3 (batch),
hg = core%2 (head-group of 8 heads). Each core computes the partial c_proj
output for its 8 heads of its batch; host adds the two per-batch partials
and the bias.

Per-core layout (all matmul operands viewed as float32r):
  x    resident [128, 2048] x8 (c on partitions), loaded once
  qt/kt pair tiles [128, 2048] x4: rows 0-63 head 2a, 64-127 head 2a+1
  v    natural [t, d] in 16 tiles [128, 8*65] with a ones column per head
  pass1: S[tq,tk] blocks directly off qt/kt (K=64, base partition 64*hip)
         -> per-row max batched into nm[128,16], one PE transpose per head
  pass2: S'[tk,tq] blocks via K=65 aug matmuls (kaug row64=ones,
         qaug row64=-max) -> exp on ScalarE -> p^T in SBUF
  PV:    psum_y[65, 512] += v_aug^T.T @ p^T  (row 64 = softmax sums, free)
  norm:  per-pair: fast reciprocal of sums, selector-matmul broadcast,
         two DVE multiplies per slice
  proj:  outT[o,t] = wpT.T @ yT  accumulated over 4 c-tiles
"""

import math
from contextlib import ExitStack

import numpy as np

import concourse.bass as bass
import concourse.bacc as bacc
import concourse.mybir as mybir
import concourse.tile as tile

F32 = mybir.dt.float32
F32R = mybir.dt.float32r
BF16 = mybir.dt.bfloat16
AF = mybir.ActivationFunctionType
AX = mybir.AxisListType
ALU = mybir.AluOpType

NEG_BIG = -30000.0


def _R(ap):
    return ap.bitcast(F32R)


def build_nc(T=2048, CK=1024, NH=8):
    """Emit the per-core program. T: seq len, CK: embed dim (contraction),
    NH: heads on this core (head dim fixed 64)."""
    HD = 64
    NP = NH // 2          # head pairs
    CL = NH * HD          # core-local channels (512)
    NT = T // 128         # tq/tk tiles
    NS = T // 512         # 512-wide slices
    NC = CK // 128        # contraction tiles

    nc = bacc.Bacc(None, target_bir_lowering=False)

    xT = nc.declare_dram_parameter("xT", [CK, T], F32, isOutput=False)
    wqT = nc.declare_dram_parameter("wqT", [CK, CL], F32, isOutput=False)
    wkT = nc.declare_dram_parameter("wkT", [CK, CL], F32, isOutput=False)
    wvT = nc.declare_dram_parameter("wvT", [CK, CL], F32, isOutput=False)
    wpT = nc.declare_dram_parameter("wpT", [CL, CK], BF16, isOutput=False)
    maskUb = nc.declare_dram_parameter("maskUb", [128, 128], BF16, isOutput=False)
    maskLb = nc.declare_dram_parameter("maskLb", [128, 128], BF16, isOutput=False)
    ident = nc.declare_dram_parameter("ident", [128, 128], F32, isOutput=False)
    identb = nc.declare_dram_parameter("identb", [128, 128], BF16, isOutput=False)
    ones_row = nc.declare_dram_parameter("ones_row", [1, T], F32, isOutput=False)
    ones_va = nc.declare_dram_parameter("ones_va", [128, NH * 128], BF16, isOutput=False)
    sel_all = nc.declare_dram_parameter("sel_all", [128, 512], F32, isOutput=False)
    zeros_pad = nc.declare_dram_parameter("zeros_pad", [120, T], F32, isOutput=False)
    outT = nc.declare_dram_parameter("outT", [CK, T], F32, isOutput=True)

    with tile.TileContext(nc) as tc, ExitStack() as ctx:
        singles = ctx.enter_context(tc.tile_pool(name="singles", bufs=1))

        ident_sb = singles.tile([128, 128], F32, tag="ident")
        nc.sync.dma_start(out=ident_sb, in_=ident[:, :])
        identR_sb = singles.tile([128, 128], F32, tag="identR")
        nc.sync.dma_start(out=_R(identR_sb), in_=_R(ident[:, :]))
        maskU_sb = singles.tile([128, 128], BF16, tag="maskU")
        nc.sync.dma_start(out=maskU_sb, in_=maskUb[:, :])
        maskL_sb = singles.tile([128, 128], BF16, tag="maskL")
        nc.sync.dma_start(out=maskL_sb, in_=maskLb[:, :])
        identb_sb = singles.tile([128, 128], BF16, tag="identb")
        nc.sync.dma_start(out=identb_sb, in_=identb[:, :])
        sel_sb = singles.tile([128, 512], F32, tag="sel")
        nc.sync.dma_start(out=_R(sel_sb), in_=_R(sel_all[:, :]))

        # resident activations
        qt = [singles.tile([128, T], F32, tag=f"qt{a}", name=f"qt{a}") for a in range(NP)]
        kt = [singles.tile([128, T], F32, tag=f"kt{a}", name=f"kt{a}") for a in range(NP)]
        va = [singles.tile([128, NH * 128], BF16, tag=f"va{i}", name=f"va{i}") for i in range(NT)]

        for i in range(NT):
            nc.sync.dma_start(out=va[i][:, :], in_=ones_va[:, :])

        wp_sb = [singles.tile([128, CK], BF16, tag=f"wp{t_}", name=f"wp{t_}")
                 for t_ in range(NP)]
        for t_ in range(NP):
            nc.sync.dma_start(out=wp_sb[t_], in_=wpT[128 * t_:128 * (t_ + 1), :])

        # ---------------- phase P: projections ----------------
        # x resident across both sub-phases (loaded once), freed before A
        with tc.tile_pool(name="xpool", bufs=1) as xpool, \
             tc.tile_pool(name="ppj", bufs=4, space="PSUM") as ppj:
            xsb = [xpool.tile([128, T], F32, tag=f"x{c}", name=f"x{c}")
                   for c in range(NC)]
            with tc.tile_pool(name="wpool1", bufs=1) as wpool:
                wq_sb = [wpool.tile([128, CL], F32, tag=f"wq{c}", name=f"wq{c}")
                         for c in range(NC)]
                wk_sb = [wpool.tile([128, CL], F32, tag=f"wk{c}", name=f"wk{c}")
                         for c in range(NC)]
                for c in range(NC):
                    nc.sync.dma_start(out=_R(wq_sb[c]), in_=_R(wqT[128 * c:128 * (c + 1), :]))
                for c in range(NC):
                    nc.sync.dma_start(out=_R(xsb[c][:, 0:512]),
                                      in_=_R(xT[128 * c:128 * (c + 1), 0:512]))
                for c in range(NC):
                    nc.sync.dma_start(out=_R(wk_sb[c]), in_=_R(wkT[128 * c:128 * (c + 1), :]))
                for half in (1, 2, 3):
                    hs = slice(512 * half, 512 * (half + 1))
                    for c in range(NC):
                        nc.sync.dma_start(out=_R(xsb[c][:, hs]),
                                          in_=_R(xT[128 * c:128 * (c + 1), hs]))
                for a in range(NP):
                    for s in range(NS):
                        for w_sb, dest in ((wq_sb, qt), (wk_sb, kt)):
                            ps = ppj.tile([128, 512], F32, tag="pj", name="pj_qk")
                            for c in range(NC):
                                nc.tensor.matmul(
                                    ps, _R(w_sb[c][:, 128 * a:128 * (a + 1)]),
                                    _R(xsb[c][:, 512 * s:512 * (s + 1)]),
                                    start=(c == 0), stop=(c == NC - 1))
                            nc.scalar.copy(_R(dest[a][:, 512 * s:512 * (s + 1)]), ps)

            with tc.tile_pool(name="wpool2", bufs=1) as wpool:
                wv_sb = [wpool.tile([128, CL], F32, tag=f"wv{c}", name=f"wv{c}")
                         for c in range(NC)]
                for c in range(NC):
                    nc.sync.dma_start(out=_R(wv_sb[c]), in_=_R(wvT[128 * c:128 * (c + 1), :]))
                for i in range(NT):
                    ps = ppj.tile([128, CL], F32, tag="pj", name="pj_v")
                    for c in range(NC):
                        nc.tensor.matmul(
                            ps, _R(xsb[c][:, 128 * i:128 * (i + 1)]), _R(wv_sb[c]),
                            start=(c == 0), stop=(c == NC - 1))
                    va_view = va[i].rearrange("p (h e) -> p h e", e=128)
                    ps_view = ps.rearrange("p (h e) -> p h e", e=HD)
                    nc.vector.tensor_copy(va_view[:, :, 0:HD], ps_view)

        # ---------------- phase A: attention ----------------
        ytpool = ctx.enter_context(tc.tile_pool(name="ytpool", bufs=1))
        yt = [ytpool.tile([128, T], BF16, tag=f"yt{a}", name=f"yt{a}") for a in range(NP)]
        su8s = [None] * NP
        rinv8s = [None] * NP
        rinv8rs = [None] * NP
        with tc.tile_pool(name="p1ps", bufs=3, space="PSUM") as p1ps, \
             tc.tile_pool(name="tpp", bufs=1, space="PSUM") as tpp, \
             tc.tile_pool(name="p2ps", bufs=2, space="PSUM") as p2ps, \
             tc.tile_pool(name="yvps", bufs=2, space="PSUM") as yvps, \
             tc.tile_pool(name="augs", bufs=2) as augs, \
             tc.tile_pool(name="ptp", bufs=3) as ptp, \
             tc.tile_pool(name="small", bufs=2) as small:

            def head_setup(h):
                """Per-head pass-2 tiles: kaug row64 = ones; qaug row64 is
                filled with -max by p1_done's DMA."""
                a, hip = h // 2, h % 2
                kaug = augs.tile([128, T], F32, tag="kaug", name=f"kaug{h}")
                qaug = augs.tile([128, T], F32, tag="qaug", name=f"qaug{h}")
                for c4 in range(4):
                    cs = slice(512 * c4, 512 * (c4 + 1))
                    nc.sync.dma_start(out=_R(kaug[0:64, cs]),
                                      in_=_R(kt[a][64 * hip:64 * hip + 64, cs]))
                    nc.sync.dma_start(out=_R(kaug[65:128, cs]),
                                      in_=_R(zeros_pad[0:63, cs]))
                    nc.sync.dma_start(out=_R(qaug[0:64, cs]),
                                      in_=_R(qt[a][64 * hip:64 * hip + 64, cs]))
                    nc.sync.dma_start(out=_R(qaug[64:128, cs]),
                                      in_=_R(zeros_pad[0:64, cs]))
                nc.sync.dma_start(out=_R(kaug[64:65, :]), in_=_R(ones_row[:, :]))
                nm = small.tile([128, 16], F32, tag="nm", name=f"nm{h}")
                tps16 = small.tile([16, 128], F32, tag="tps16", name=f"tps16{h}")
                if hip == 0:
                    su8s[a] = small.tile([8, 512], F32, tag="su8",
                                         name=f"su8_{a}")
                    rinv8s[a] = small.tile([8, 512], F32, tag="rinv8",
                                           name=f"rinv8_{a}")
                    rinv8rs[a] = small.tile([128, 512], F32, tag="rinv8r",
                                            name=f"rinv8r_{a}")
                    nc.sync.dma_start(out=_R(rinv8rs[a][8:128, :]),
                                      in_=_R(zeros_pad[0:120, 0:512]))
                ytmp = [small.tile([65, 512], F32, tag=f"ytmp{hip}_{s}",
                                   name=f"ytmp{h}_{s}", bufs=1) for s in range(NS)]
                return dict(h=h, kaug=kaug, qaug=qaug, nm=nm, tps16=tps16,
                            ytmp=ytmp)

            def p1_row(st, i):
                """Pass-1 row i for head h: causal-trimmed S blocks off the
                padded aug tiles (stationary full [128,128]; qaug row 64 is
                still zero here so kaug's ones row contributes nothing);
                -max of each row into nm[:, i]."""
                h, nm = st["h"], st["nm"]
                kaug, qaug = st["kaug"], st["qaug"]
                jd, m = i // 4, i % 4
                W = 128 * (m + 1)            # diag block valid width
                mA = small.tile([128, 4], F32, tag="mA", name="mA")
                for j in range(jd + 1):
                    diag = (j == jd)
                    w = W if diag else 512
                    psA = p1ps.tile([128, 512], F32, tag="blk", name="psA")
                    nc.tensor.matmul(
                        psA[:, 0:w], _R(qaug[:, 128 * i:128 * (i + 1)]),
                        _R(kaug[:, 512 * j:512 * j + w]),
                        start=True, stop=not diag)
                    if diag:
                        # boundary 128-chunk gets the pure triangular mask
                        nc.tensor.matmul(psA[:, W - 128:W], identb_sb,
                                         maskU_sb, start=False, stop=True)
                    nc.vector.reduce_max(mA[:, j:j + 1], psA[:, 0:w], axis=AX.X)
                nc.vector.tensor_reduce(
                    nm[:, i:i + 1], mA[:, 0:jd + 1], axis=AX.X, op=ALU.max,
                    negate=True)

            def p1_done(st):
                # batched transpose of the 16 per-row negmax columns, then one
                # DMA drops them into qaug row 64 as the pass-2 bias row
                h, nm, tps16 = st["h"], st["nm"], st["tps16"]
                tp = tpp.tile([16, 128], F32, tag="tp", name="tp")
                nc.tensor.transpose(tp, nm, ident_sb)
                nc.vector.tensor_copy(tps16, tp)
                nc.sync.dma_start(out=_R(st["qaug"][64:65, :]), in_=_R(tps16))

            def a2_slice(st, s):
                """Pass-2 + PV for (head, tq-slice s), causal-trimmed."""
                h, kaug, qaug = st["h"], st["kaug"], st["qaug"]
                hip = h % 2
                nts = 4 * s + 4
                yps = yvps.tile([128, 512], F32, tag="y", name="yps")
                for t in range(nts):
                    mp = t - 4 * s
                    c0 = 128 * mp if mp > 0 else 0   # first valid column
                    ps2 = p2ps.tile([128, 512], F32, tag="s2", name="ps2")
                    nc.tensor.matmul(
                        ps2[:, c0:512], _R(kaug[:, 128 * t:128 * (t + 1)]),
                        _R(qaug[:, 512 * s + c0:512 * (s + 1)]),
                        start=True, stop=(mp < 0))
                    if mp >= 0:
                        nc.tensor.matmul(ps2[:, c0:c0 + 128], identb_sb,
                                         maskL_sb, start=False, stop=True)
                    pt = ptp.tile([128, 512], BF16, tag="pt", name="pt")
                    nc.scalar.activation(pt[:, c0:512], ps2[:, c0:512], AF.Exp)
                    nc.tensor.matmul(
                        yps[:, c0:512], va[t][:, 128 * h:128 * h + 128],
                        pt[:, c0:512],
                        start=(t == 0), stop=(t == nts - 1),
                        skip_group_check=True)
                nc.scalar.copy(st["ytmp"][s], yps[0:65, :])
                a = h // 2
                nc.sync.dma_start(out=su8s[a][4 * hip + s:4 * hip + s + 1, :],
                                  in_=st["ytmp"][s][64:65, :])

            def a2_recip(st):
                h = st["h"]
                a, hip = h // 2, h % 2
                if hip == 1:
                    nc.vector.reciprocal_approx_fast(rinv8s[a], su8s[a])
                    nc.vector.tensor_copy(_R(rinv8rs[a][0:8, :]), rinv8s[a])

            def pair_finish(st_lo, st_hi, s):
                """Scale both heads' slice s by 1/sum and write into yt."""
                h = st_hi["h"]
                a = h // 2
                sc = yvps.tile([128, 512], F32, tag="y", name="sc")
                nc.tensor.matmul(sc, _R(sel_sb[:, 128 * s:128 * (s + 1)]),
                                 _R(rinv8rs[a]), start=True, stop=True)
                nc.vector.tensor_mul(
                    yt[a][0:64, 512 * s:512 * (s + 1)],
                    st_lo["ytmp"][s][0:64, :], sc[0:64, :])
                nc.vector.tensor_mul(
                    yt[a][64:128, 512 * s:512 * (s + 1)],
                    st_hi["ytmp"][s][0:64, :], sc[64:128, :])

            # software pipeline at head granularity: pass-1 of head h runs
            # (PE-light, DVE-heavy) interleaved with pass-2/PV of head h-1
            # (PE-heavy) so both engines stay fed.
            def a2_sections(st):
                for s in range(NS):
                    yield lambda st=st, s=s: a2_slice(st, s)
                yield lambda st=st: a2_recip(st)
                if st["h"] % 2 == 1:
                    lo = sts[st["h"] - 1]
                    for s in range(NS):
                        yield lambda lo=lo, st=st, s=s: pair_finish(lo, st, s)

            sts = [None] * NH
            for h in range(NH + 1):
                prev_iter = a2_sections(sts[h - 1]) if h >= 1 else iter(())
                if h < NH:
                    sts[h] = head_setup(h)
                    for i in range(NT):
                        p1_row(sts[h], i)
                        if i % 3 == 2:
                            nxt = next(prev_iter, None)
                            if nxt is not None:
                                nxt()
                for nxt in prev_iter:
                    nxt()
                if h < NH:
                    p1_done(sts[h])

        # ---------------- phase PR: output projection ----------------
        with tc.tile_pool(name="prps", bufs=4, space="PSUM") as prps, \
             tc.tile_pool(name="stg", bufs=3) as stg:
            for o in range(NC):
                for s in range(NS):
                    ps = prps.tile([128, 512], F32, tag="pr")
                    for t_ in range(NP):
                        nc.tensor.matmul(
                            ps, wp_sb[t_][:, 128 * o:128 * (o + 1)],
                            yt[t_][:, 512 * s:512 * (s + 1)],
                            start=(t_ == 0), stop=(t_ == NP - 1))
                    st = stg.tile([128, 512], F32, tag="st")
                    nc.scalar.copy(st, ps)
                    nc.sync.dma_start(
                        out=outT[128 * o:128 * (o + 1), 512 * s:512 * (s + 1)], in_=st)

    nc.finalize()
    return nc


def make_masks():
    from ml_dtypes import bfloat16
    r = np.arange(128)[:, None]
    c = np.arange(128)[None, :]
    maskU = np.where(c > r, NEG_BIG, 0.0).astype(bfloat16)
    maskL = np.where(c < r, NEG_BIG, 0.0).astype(bfloat16)
    ident = np.eye(128, dtype=np.float32)
    identb = np.eye(128, dtype=np.float32).astype(bfloat16)
    return maskU, maskL, ident, identb


def make_sel():
    # sel_all[r, 128*s + p]: block s row (4*(p>=64) + s) is one
    sel = np.zeros((128, 512), dtype=np.float32)
    for s in range(4):
        sel[s, 128 * s:128 * s + 64] = 1.0
        sel[4 + s, 128 * s + 64:128 * (s + 1)] = 1.0
    return sel


def make_in_maps(x, W_attn, W_proj, n_cores=8, NH=8):
    from ml_dtypes import bfloat16
    maskU, maskL, ident, identb = make_masks()
    sel = make_sel()
    T = x.shape[1]
    ones_va = np.zeros((128, NH * 128), dtype=bfloat16)
    ones_va[:, 64::128] = 1.0
    in_maps = []
    for core in range(n_cores):
        b, hg = core // 2, core % 2
        CL = NH * 64
        r0 = hg * CL
        C = x.shape[2]
        wq = np.ascontiguousarray((8.0 * W_attn[r0:r0 + CL, :]).T)
        wk = np.ascontiguousarray(W_attn[C + r0:C + r0 + CL, :].T)
        wv = np.ascontiguousarray(W_attn[2 * C + r0:2 * C + r0 + CL, :].T)
        wp = np.ascontiguousarray(W_proj[:, r0:r0 + CL].T).astype(bfloat16)
        in_maps.append({
            "xT": np.ascontiguousarray(x[b].T),
            "wqT": wq, "wkT": wk, "wvT": wv, "wpT": wp,
            "maskUb": maskU, "maskLb": maskL, "ident": ident,
            "identb": identb,
            "ones_row": np.ones((1, T), dtype=np.float32),
            "ones_va": ones_va,
            "sel_all": sel,
            "zeros_pad": np.zeros((120, T), dtype=np.float32),
        })
    return in_maps


last_results = None


def kernel(x, W_attn, W_proj, b_proj):
    global last_results
    from concourse.bass_utils import run_bass_kernel_spmd

    x = np.asarray(x, dtype=np.float32)
    W_attn = np.asarray(W_attn, dtype=np.float32)
    W_proj = np.asarray(W_proj, dtype=np.float32)
    b_proj = np.asarray(b_proj, dtype=np.float32)

    nc = build_nc(T=2048, CK=1024, NH=8)
    in_maps = make_in_maps(x, W_attn, W_proj)
    res = run_bass_kernel_spmd(nc, in_maps, list(range(8)))
    last_results = res
    outs = []
    for b in range(4):
        o = res.results[2 * b]["outT"] + res.results[2 * b + 1]["outT"]
        outs.append(o.T + b_proj[None, :])
    return np.stack(outs).astype(np.float32)
